# revision 6
# baseline (speedup 1.0000x reference)
"""Trainium2 Bass kernel for nn_DigitConvolutionalModel (dense CNN -> MLP).

Pure data parallel over 8 NeuronCores (2048 samples each). The 3x3 conv is
linear, so the host folds it into the first FC layer (W1e = C @ w1.T), making
the whole network a 4-layer MLP computed in transposed orientation (features
on partitions, batch on the free dim) in fp16 (psum fp32, ~5e-4 rel err):

    outT = w4t.T @ relu(w3t.T @ relu(w2t.T @ relu(W1e.T @ xT + b1) + b2) + b3) + b4

v2 structure (vs the single-queue baseline):
  - Input DMA split over THREE parallel queues: SP (sync, HWDGE),
    ACT (scalar, HWDGE) and Pool (gpsimd, SWDGE); each x tile split into
    chunk groups c0-2 / c3-4 / c5-6, one group per queue, tile-ordered.
  - PE warmup matmuls sized to keep PE busy until the first x chunks land
    (HAM flips to 2.4 GHz during A0 instead of during A2).
  - PE tail ops pulled forward: C1 B2 between A3's m-groups, only the
    B3->C3->D3 chain of the last tile is exposed after A3.
  - All kernel semaphores pinned to nums 208+ (the Sync engine's chunk of
    walrus's end-of-NEFF semaphore-zeroing epilogue) and the final
    out-DMA wait moved AFTER the block barrier, so the ~6us per-engine
    zeroing chains overlap the out-DMA completion instead of following it.

PE op order (A=L1 m-group, B=L2, C=L3, D=L4):
  A0m0 A0m1 A1m0 A1m1 B0 A2m0 C0 A2m1 B1 D0 A3m0 C1 B2 A3m1 D1 C2 B3 D2 C3 D3
ACT: r00 r01 r10 r11 r20 r21 h3(0) r30 r31 h3(1) h3(2) h3(3)   (sa +1 each)
DVE: h2(0) h2(1) out(0) h2(2) out(1) h2(3) out(2) out(3)       (sv +1 each)
s2 counts PE tail ops (B/C/D) in PE order.
"""

from contextlib import ExitStack

import ml_dtypes
import numpy as np

import concourse.bass as bass
import concourse.mybir as mybir

N_CORES = 8
B = 16384
BC = B // N_CORES
NB = 512
NT = BC // NB
KC = 112
NKC = 7

F32 = mybir.dt.float32
BF16 = mybir.dt.bfloat16
FP16 = mybir.dt.float16
RELU = mybir.ActivationFunctionType.Relu
ADD = mybir.AluOpType.add
MAX = mybir.AluOpType.max

N_WARM_MM = 7

# Tile-0 x chunk splits (fine-grained so A0 starts as chunks land) and the
# w1e split, all on the single sync DMA queue in need order.  Tiles 1-3 are
# one 802KB DMA each (bigger transfers run closer to peak DMA rate).
X0_SPLITS = [(0, 1), (1, 2), (2, 4), (4, 7)]
W1A = (0, 3)
W1B = (3, 7)

# PE tail-op order; s2 threshold = 1-based position.
TAIL_ORDER = [
    ("B", 0), ("C", 0), ("B", 1), ("D", 0), ("C", 1), ("B", 2),
    ("D", 1), ("C", 2), ("B", 3), ("D", 2), ("C", 3), ("D", 3),
]
POS_PE = {op: i + 1 for i, op in enumerate(TAIL_ORDER)}

ACT_ORDER = [
    ("r", 0, 0), ("r", 0, 1), ("r", 1, 0), ("r", 1, 1), ("r", 2, 0),
    ("r", 2, 1), ("h3", 0), ("r", 3, 0), ("r", 3, 1), ("h3", 1),
    ("h3", 2), ("h3", 3),
]
POS_A = {op: i + 1 for i, op in enumerate(ACT_ORDER)}

DVE_ORDER = [
    ("h2", 0), ("h2", 1), ("out", 0), ("h2", 2), ("out", 1), ("h2", 3),
    ("out", 2), ("out", 3),
]
POS_V = {op: i + 1 for i, op in enumerate(DVE_ORDER)}

# Explicit semaphore numbers inside [208, 255]: the chunk of walrus's
# end-of-NEFF zeroing epilogue that the Sync engine clears LAST (after its
# own final instruction, i.e. after the post-block sof wait).  Any sem a
# post-barrier wait depends on must live here.
SEM_BASE = 208


def build_program(l1_dt=FP16, l234_dt=FP16):
    nc = bass.Bass()

    n_wp = 256 + 64 + 10

    xt_d = nc.declare_dram_parameter("xt", [NT, KC, NKC * NB], l1_dt, isOutput=False)
    w1_d = nc.declare_dram_parameter("w1e", [KC, NKC * 256], l1_dt, isOutput=False)
    wp_d = nc.declare_dram_parameter("wpack", [128, n_wp], l234_dt, isOutput=False)
    bp_d = nc.declare_dram_parameter("bpack", [128, 5], F32, isOutput=False)
    out_d = nc.declare_dram_parameter("outT", [10, BC], F32, isOutput=True)

    ctx = ExitStack()
    with ctx:
        xsb = ctx.enter_context(nc.sbuf_tensor([KC, NT, NKC, NB], l1_dt))
        w1sb = ctx.enter_context(nc.sbuf_tensor([KC, NKC, 256], l1_dt))
        wpsb = ctx.enter_context(nc.sbuf_tensor([128, n_wp], l234_dt))
        bpsb = ctx.enter_context(nc.sbuf_tensor([128, 5], F32))
        h1sb = ctx.enter_context(nc.sbuf_tensor([128, 2, 2, NB], l234_dt))
        h2sb = ctx.enter_context(nc.sbuf_tensor([128, 2, NB], l234_dt))
        h3sb = ctx.enter_context(nc.sbuf_tensor([64, 2, NB], l234_dt))
        osb = ctx.enter_context(nc.sbuf_tensor([10, NT, NB], F32))
        warm = ctx.enter_context(nc.sbuf_tensor([1, 513], BF16))
        dump_a = ctx.enter_context(nc.sbuf_tensor([1, 16], BF16))
        dump_v = ctx.enter_context(nc.sbuf_tensor([1, 16], BF16))

        w2v = wpsb[:, 0:256].rearrange("p (c o) -> p c o", c=2)
        w3v = wpsb[:, 256:320]
        w4v = wpsb[0:64, 320:330]
        b1v = bpsb[:, 0:2]
        b2v = bpsb[:, 2:3]
        b3v = bpsb[0:64, 3:4]
        b4v = bpsb[0:10, 4:5]

        ps1 = ctx.enter_context(nc.psum_tensor([128, 2, 2, NB], F32))
        ps2 = ctx.enter_context(nc.psum_tensor([128, NB], F32))
        ps3 = ctx.enter_context(nc.psum_tensor([64, NB], F32))
        ps4 = ctx.enter_context(nc.psum_tensor([10, NB], F32))

        nsem = iter(range(SEM_BASE, 256))

        def sem(name):
            return ctx.enter_context(nc.semaphore(name, num=next(nsem)))

        sg = sem("sg")
        swr = sem("swr")
        sw1a = sem("sw1a")
        sw1b = sem("sw1b")
        sx0 = [sem(f"sx0_{g}") for g in range(len(X0_SPLITS))]
        sxt = [None] + [sem(f"sx{t}") for t in range(1, NT)]
        sm = sem("sm")
        s2 = sem("s2")
        sa = sem("sa")
        sv = sem("sv")
        sof = sem("sof")

        with nc.Block() as block:

            @block.sync
            def _(sy):
                # Single need-ordered queue: w1/x interleaved so A0 can start
                # on partial tile-0 data, tiles 1-3 as one big DMA each.
                a0, a1 = W1A
                sy.dma_start(
                    out=w1sb[:, a0:a1, :], in_=w1_d[:, a0 * 256 : a1 * 256]
                ).then_inc(sw1a, 16)
                for g in (0, 1):
                    c0, c1 = X0_SPLITS[g]
                    sy.dma_start(
                        out=xsb[:, 0, c0:c1, :], in_=xt_d[0, :, c0 * NB : c1 * NB]
                    ).then_inc(sx0[g], 16)
                b0, b1 = W1B
                sy.dma_start(
                    out=w1sb[:, b0:b1, :], in_=w1_d[:, b0 * 256 : b1 * 256]
                ).then_inc(sw1b, 16)
                for g in (2, 3):
                    c0, c1 = X0_SPLITS[g]
                    sy.dma_start(
                        out=xsb[:, 0, c0:c1, :], in_=xt_d[0, :, c0 * NB : c1 * NB]
                    ).then_inc(sx0[g], 16)
                for t in range(1, NT):
                    sy.dma_start(
                        out=xsb[:, t, :, :], in_=xt_d[t, :, :]
                    ).then_inc(sxt[t], 16)
                for t in range(NT):
                    sy.wait_ge(sv, POS_V[("out", t)])
                    sy.dma_start(
                        out=out_d[:, t * NB : (t + 1) * NB], in_=osb[:, t, :]
                    ).then_inc(sof, 16)

            @block.scalar
            def _(se):
                se.wait_ge(sg, 1)
                se.activation(dump_a[:], warm[:, 0:16], RELU)  # preload relu table
                se.wait_ge(swr, 32)
                for op in ACT_ORDER:
                    if op[0] == "r":
                        _, t, m = op
                        st = t % 2
                        if t >= 2:
                            # h1sb[st] freed once B(t-2) consumed it
                            se.wait_ge(s2, POS_PE[("B", t - 2)])
                        se.wait_ge(sm, 2 * t + m + 1)
                        se.activation(
                            h1sb[:, st, m, :], ps1[:, st, m, :], RELU,
                            bias=b1v[:, m : m + 1],
                        ).then_inc(sa, 1)
                    else:
                        _, t = op
                        st = t % 2
                        se.wait_ge(s2, POS_PE[("C", t)])
                        se.activation(
                            h3sb[:, st, :], ps3[:], RELU, bias=b3v[:]
                        ).then_inc(sa, 1)

            @block.gpsimd
            def _(ge):
                # SWDGE side queue for the tiny mlp weights/biases — keeps
                # their issue+transfer off the x stream's critical path.
                ge.dma_start(out=wpsb[:], in_=wp_d[:]).then_inc(swr, 16)
                ge.dma_start(out=bpsb[:], in_=bp_d[:]).then_inc(swr, 16)

            @block.vector
            def _(ve):
                ve.memset(warm[:], 0.125).then_inc(sg, 1)
                ve.wait_ge(sg, 1)
                ve.tensor_scalar(dump_v[:], warm[:, 0:16], 0.0, 0.0, ADD, MAX)
                ve.wait_ge(swr, 32)
                for kind, t in DVE_ORDER:
                    st = t % 2
                    if kind == "h2":
                        ve.wait_ge(s2, POS_PE[("B", t)])
                        ve.tensor_scalar(
                            h2sb[:, st, :], ps2[:], b2v[:], 0.0, ADD, MAX
                        ).then_inc(sv, 1)
                    else:
                        ve.wait_ge(s2, POS_PE[("D", t)])
                        ve.tensor_scalar(
                            osb[:, t, :], ps4[:], b4v[:], None, ADD
                        ).then_inc(sv, 1)

            @block.tensor
            def _(te):
                te.wait_ge(sg, 1)
                for _i in range(N_WARM_MM):
                    te.matmul(ps2[0:1, :], warm[:, 0:1], warm[:, 1:513],
                              start=True, stop=True)

                def emit_A(t, m):
                    st = t % 2
                    if t >= 2:
                        # ps1[st, m] freed once r(t-2, m) consumed it
                        te.wait_ge(sa, POS_A[("r", t - 2, m)])
                    for c in range(NKC):
                        if m == 0:
                            if t == 0:
                                for g, (a, _b) in enumerate(X0_SPLITS):
                                    if a == c:
                                        te.wait_ge(sx0[g], 16)
                                if c == W1A[0]:
                                    te.wait_ge(sw1a, 16)
                                if c == W1B[0]:
                                    te.wait_ge(sw1b, 16)
                            elif c == 0:
                                te.wait_ge(sxt[t], 16)
                        mm = te.matmul(
                            ps1[:, st, m, :],
                            w1sb[:, c, m * 128 : (m + 1) * 128],
                            xsb[:, t, c, :],
                            start=(c == 0),
                            stop=(c == NKC - 1),
                        )
                        if c == NKC - 1:
                            mm.then_inc(sm, 1)

                def emit_B(t):
                    st = t % 2
                    if t == 0:
                        te.wait_ge(swr, 32)
                    if t >= 1:
                        te.wait_ge(sv, POS_V[("h2", t - 1)])  # ps2 free
                    te.wait_ge(sa, POS_A[("r", t, 0)])
                    te.matmul(
                        ps2[:], w2v[:, 0, :], h1sb[:, st, 0, :],
                        start=True, stop=False,
                    )
                    te.wait_ge(sa, POS_A[("r", t, 1)])
                    te.matmul(
                        ps2[:], w2v[:, 1, :], h1sb[:, st, 1, :],
                        start=False, stop=True,
                    ).then_inc(s2, 1)

                def emit_C(t):
                    st = t % 2
                    if t >= 1:
                        te.wait_ge(sa, POS_A[("h3", t - 1)])  # ps3 free
                    te.wait_ge(sv, POS_V[("h2", t)])
                    te.matmul(
                        ps3[:], w3v[:], h2sb[:, st, :], start=True, stop=True
                    ).then_inc(s2, 1)

                def emit_D(t):
                    st = t % 2
                    if t >= 1:
                        te.wait_ge(sv, POS_V[("out", t - 1)])  # ps4 free
                    te.wait_ge(sa, POS_A[("h3", t)])
                    te.matmul(
                        ps4[:], w4v[:], h3sb[:, st, :], start=True, stop=True
                    ).then_inc(s2, 1)

                emit_A(0, 0)
                emit_A(0, 1)
                emit_A(1, 0)
                emit_A(1, 1)
                emit_B(0)
                emit_A(2, 0)
                emit_C(0)
                emit_A(2, 1)
                emit_B(1)
                emit_D(0)
                emit_A(3, 0)
                emit_C(1)
                emit_B(2)
                emit_A(3, 1)
                emit_D(1)
                emit_C(2)
                emit_B(3)
                emit_D(2)
                emit_C(3)
                emit_D(3)

        # Post-block: only Sync still has work (awaiting out-DMA landing in
        # HBM).  Walrus's per-engine semaphore-zeroing epilogue chains start
        # right after each engine's barrier release, overlapping this wait.
        nc.sync.wait_ge(sof, 16 * NT)

    return nc


def _np_dt(dt):
    if dt == BF16:
        return ml_dtypes.bfloat16
    if dt == FP16:
        return np.float16
    return np.float32


def prepare_inputs(x, conv_w, w1, b1, w2, b2, w3, b3, w4, b4,
                   l1_dt=FP16, l234_dt=FP16):
    w1v = np.ascontiguousarray(w1.T).reshape(26, 26, 256)
    w1e = np.zeros((28, 28, 256), dtype=np.float32)
    for di in range(3):
        for dj in range(3):
            w1e[di : di + 26, dj : dj + 26, :] += conv_w[di, dj] * w1v
    w1e = w1e.reshape(784, 256)
    w1t = np.ascontiguousarray(
        w1e.reshape(NKC, KC, 256).transpose(1, 0, 2)
    ).reshape(KC, NKC * 256).astype(_np_dt(l1_dt))

    w2t = np.ascontiguousarray(w2.T).reshape(2, 128, 128).transpose(1, 0, 2)
    wpack = np.zeros((128, 256 + 64 + 10), dtype=np.float32)
    wpack[:, 0:256] = w2t.reshape(128, 256)
    wpack[:, 256:320] = w3.T
    wpack[0:64, 320:330] = w4.T
    wpack = wpack.astype(_np_dt(l234_dt))

    bpack = np.zeros((128, 5), dtype=np.float32)
    bpack[:, 0:2] = b1.reshape(2, 128).T
    bpack[:, 2] = b2
    bpack[0:64, 3] = b3
    bpack[0:10, 4] = b4

    shared = {"w1e": w1t, "wpack": wpack, "bpack": bpack}
    in_maps = []
    for m in range(N_CORES):
        xc = x[m * BC : (m + 1) * BC]
        xt = np.ascontiguousarray(
            xc.reshape(NT, NB, NKC, KC).transpose(0, 3, 2, 1)
        ).reshape(NT, KC, NKC * NB).astype(_np_dt(l1_dt))
        in_maps.append({"xt": xt, **shared})
    return in_maps



_PROGRAM = None


def _get_program():
    global _PROGRAM
    if _PROGRAM is None:
        _PROGRAM = build_program()
    return _PROGRAM


def kernel(x, conv_w, w1, b1, w2, b2, w3, b3, w4, b4):
    from concourse import bass_utils

    args = [x, conv_w, w1, b1, w2, b2, w3, b3, w4, b4]
    x, conv_w, w1, b1, w2, b2, w3, b3, w4, b4 = [
        np.asarray(a, dtype=np.float32) for a in args
    ]
    nc = _get_program()
    in_maps = prepare_inputs(x, conv_w, w1, b1, w2, b2, w3, b3, w4, b4)
    res = bass_utils.run_bass_kernel_spmd(nc, in_maps, list(range(N_CORES)))
    out = np.concatenate(
        [np.ascontiguousarray(res.results[m]["outT"].T) for m in range(N_CORES)],
        axis=0,
    )
    return out.astype(np.float32)


# revision 10
# speedup vs baseline: 1.0566x; 1.0566x over previous
"""Trainium2 Bass kernel for nn_DigitConvolutionalModel (dense CNN -> MLP).

Pure data parallel over 8 NeuronCores (2048 samples each). The 3x3 conv is
linear, so the host folds it into the first FC layer (W1e = C @ w1.T), making
the whole network a 4-layer MLP computed in transposed orientation (features
on partitions, batch on the free dim) in fp16 (psum fp32, ~5e-4 rel err):

    outT = w4t.T @ relu(w3t.T @ relu(w2t.T @ relu(W1e.T @ xT + b1) + b2) + b3) + b4

v2 structure (vs the single-queue baseline):
  - Input DMA split over THREE parallel queues: SP (sync, HWDGE),
    ACT (scalar, HWDGE) and Pool (gpsimd, SWDGE); each x tile split into
    chunk groups c0-2 / c3-4 / c5-6, one group per queue, tile-ordered.
  - PE warmup matmuls sized to keep PE busy until the first x chunks land
    (HAM flips to 2.4 GHz during A0 instead of during A2).
  - PE tail ops pulled forward: C1 B2 between A3's m-groups, only the
    B3->C3->D3 chain of the last tile is exposed after A3.
  - All kernel semaphores pinned to nums 208+ (the Sync engine's chunk of
    walrus's end-of-NEFF semaphore-zeroing epilogue) and the final
    out-DMA wait moved AFTER the block barrier, so the ~6us per-engine
    zeroing chains overlap the out-DMA completion instead of following it.

PE op order (A=L1 m-group, B=L2, C=L3, D=L4):
  A0m0 A0m1 A1m0 A1m1 B0 A2m0 C0 A2m1 B1 D0 A3m0 C1 B2 A3m1 D1 C2 B3 D2 C3 D3
ACT: r00 r01 r10 r11 r20 r21 h3(0) r30 r31 h3(1) h3(2) h3(3)   (sa +1 each)
DVE: h2(0) h2(1) out(0) h2(2) out(1) h2(3) out(2) out(3)       (sv +1 each)
s2 counts PE tail ops (B/C/D) in PE order.
"""

from contextlib import ExitStack

import ml_dtypes
import numpy as np

import concourse.bass as bass
import concourse.mybir as mybir

N_CORES = 8
B = 16384
BC = B // N_CORES
NB = 512
NT = BC // NB
KC = 112
NKC = 7

F32 = mybir.dt.float32
BF16 = mybir.dt.bfloat16
FP16 = mybir.dt.float16
RELU = mybir.ActivationFunctionType.Relu
ADD = mybir.AluOpType.add
MAX = mybir.AluOpType.max

N_WARM_MM = 9

# x chunk splits per tile on the single sync DMA queue in need order:
# tile 0 fine-grained so A0 starts as chunks land, tiles 1-3 in two chunks.
X0_SPLITS = [(0, 1), (1, 2), (2, 4), (4, 7)]
XT_SPLITS = [(0, 4), (4, 7)]
W1A = (0, 3)
W1B = (3, 7)

# PE tail-op order; s2 threshold = 1-based position.
TAIL_ORDER = [
    ("B", 0), ("C", 0), ("B", 1), ("D", 0), ("C", 1), ("B", 2),
    ("D", 1), ("C", 2), ("B", 3), ("D", 2), ("C", 3), ("D", 3),
]
POS_PE = {op: i + 1 for i, op in enumerate(TAIL_ORDER)}

ACT_ORDER = [
    ("r", 0, 0), ("r", 0, 1), ("r", 1, 0), ("r", 1, 1), ("r", 2, 0),
    ("r", 2, 1), ("h3", 0), ("r", 3, 0), ("r", 3, 1), ("h3", 1),
    ("h3", 2), ("h3", 3),
]
POS_A = {op: i + 1 for i, op in enumerate(ACT_ORDER)}

DVE_ORDER = [
    ("h2", 0), ("h2", 1), ("out", 0), ("h2", 2), ("out", 1), ("h2", 3),
    ("out", 2), ("out", 3),
]
POS_V = {op: i + 1 for i, op in enumerate(DVE_ORDER)}

# Explicit semaphore numbers inside [208, 255]: the chunk of walrus's
# end-of-NEFF zeroing epilogue that the Sync engine clears LAST (after its
# own final instruction, i.e. after the post-block sof wait).  Any sem a
# post-barrier wait depends on must live here.
SEM_BASE = 208


def build_program(l1_dt=FP16, l234_dt=FP16):
    nc = bass.Bass()

    n_wp = 256 + 64 + 10

    xt_d = nc.declare_dram_parameter("xt", [NT, KC, NKC * NB], l1_dt, isOutput=False)
    w1_d = nc.declare_dram_parameter("w1e", [KC, NKC * 256], l1_dt, isOutput=False)
    wp_d = nc.declare_dram_parameter("wpack", [128, n_wp], l234_dt, isOutput=False)
    bp_d = nc.declare_dram_parameter("bpack", [128, 5], F32, isOutput=False)
    out_d = nc.declare_dram_parameter("outT", [10, BC], F32, isOutput=True)

    ctx = ExitStack()
    with ctx:
        xsb = ctx.enter_context(nc.sbuf_tensor([KC, NT, NKC, NB], l1_dt))
        w1sb = ctx.enter_context(nc.sbuf_tensor([KC, NKC, 256], l1_dt))
        wpsb = ctx.enter_context(nc.sbuf_tensor([128, n_wp], l234_dt))
        bpsb = ctx.enter_context(nc.sbuf_tensor([128, 5], F32))
        h1sb = ctx.enter_context(nc.sbuf_tensor([128, 2, 2, NB], l234_dt))
        h2sb = ctx.enter_context(nc.sbuf_tensor([128, 2, NB], l234_dt))
        h3sb = ctx.enter_context(nc.sbuf_tensor([64, 2, NB], l234_dt))
        osb = ctx.enter_context(nc.sbuf_tensor([10, NT, NB], F32))
        warm = ctx.enter_context(nc.sbuf_tensor([1, 513], BF16))
        dump_a = ctx.enter_context(nc.sbuf_tensor([1, 16], BF16))
        dump_v = ctx.enter_context(nc.sbuf_tensor([1, 16], BF16))

        w2v = wpsb[:, 0:256].rearrange("p (c o) -> p c o", c=2)
        w3v = wpsb[:, 256:320]
        w4v = wpsb[0:64, 320:330]
        b1v = bpsb[:, 0:2]
        b2v = bpsb[:, 2:3]
        b3v = bpsb[0:64, 3:4]
        b4v = bpsb[0:10, 4:5]

        ps1 = ctx.enter_context(nc.psum_tensor([128, 2, 2, NB], F32))
        ps2 = ctx.enter_context(nc.psum_tensor([128, NB], F32))
        ps3 = ctx.enter_context(nc.psum_tensor([64, NB], F32))
        ps4 = ctx.enter_context(nc.psum_tensor([10, NB], F32))

        nsem = iter(range(SEM_BASE, 256))

        def sem(name):
            return ctx.enter_context(nc.semaphore(name, num=next(nsem)))

        sg = sem("sg")
        swr = sem("swr")
        sw1a = sem("sw1a")
        sw1b = sem("sw1b")
        sx0 = [sem(f"sx0_{g}") for g in range(len(X0_SPLITS))]
        sxt = [None] + [
            [sem(f"sx{t}_{g}") for g in range(len(XT_SPLITS))]
            for t in range(1, NT)
        ]
        sm = sem("sm")
        s2 = sem("s2")
        sa = sem("sa")
        sv = sem("sv")
        sof = sem("sof")

        # Pre-block emission: input DMA issues and the PE warmup run right
        # after the framework preamble, ~1us before block-entry branches.
        a0, a1 = W1A
        nc.sync.dma_start(
            out=w1sb[:, a0:a1, :], in_=w1_d[:, a0 * 256 : a1 * 256]
        ).then_inc(sw1a, 16)
        for g in (0, 1):
            c0, c1 = X0_SPLITS[g]
            nc.sync.dma_start(
                out=xsb[:, 0, c0:c1, :], in_=xt_d[0, :, c0 * NB : c1 * NB]
            ).then_inc(sx0[g], 16)
        b0, b1 = W1B
        nc.sync.dma_start(
            out=w1sb[:, b0:b1, :], in_=w1_d[:, b0 * 256 : b1 * 256]
        ).then_inc(sw1b, 16)
        for g in (2, 3):
            c0, c1 = X0_SPLITS[g]
            nc.sync.dma_start(
                out=xsb[:, 0, c0:c1, :], in_=xt_d[0, :, c0 * NB : c1 * NB]
            ).then_inc(sx0[g], 16)
        for t in range(1, NT):
            for g, (c0, c1) in enumerate(XT_SPLITS):
                nc.sync.dma_start(
                    out=xsb[:, t, c0:c1, :], in_=xt_d[t, :, c0 * NB : c1 * NB]
                ).then_inc(sxt[t][g], 16)

        # warm tensor on gpsimd (fast memset) so PE needn't wait for DVE
        nc.gpsimd.memset(warm[:], 0.125).then_inc(sg, 1)
        nc.gpsimd.dma_start(out=wpsb[:], in_=wp_d[:]).then_inc(swr, 16)
        nc.gpsimd.dma_start(out=bpsb[:], in_=bp_d[:]).then_inc(swr, 16)

        with nc.Block() as block:

            @block.sync
            def _(sy):
                for t in range(NT):
                    sy.wait_ge(sv, POS_V[("out", t)])
                    sy.dma_start(
                        out=out_d[:, t * NB : (t + 1) * NB], in_=osb[:, t, :]
                    ).then_inc(sof, 16)

            @block.scalar
            def _(se):
                se.wait_ge(sg, 1)
                se.activation(dump_a[:], warm[:, 0:16], RELU)  # preload relu table
                se.wait_ge(swr, 32)
                for op in ACT_ORDER:
                    if op[0] == "r":
                        _, t, m = op
                        st = t % 2
                        if t >= 2:
                            # h1sb[st] freed once B(t-2) consumed it
                            se.wait_ge(s2, POS_PE[("B", t - 2)])
                        se.wait_ge(sm, 2 * t + m + 1)
                        se.activation(
                            h1sb[:, st, m, :], ps1[:, st, m, :], RELU,
                            bias=b1v[:, m : m + 1],
                        ).then_inc(sa, 1)
                    else:
                        _, t = op
                        st = t % 2
                        se.wait_ge(s2, POS_PE[("C", t)])
                        se.activation(
                            h3sb[:, st, :], ps3[:], RELU, bias=b3v[:]
                        ).then_inc(sa, 1)

            @block.vector
            def _(ve):
                ve.wait_ge(sg, 1)
                ve.tensor_scalar(dump_v[:], warm[:, 0:16], 0.0, 0.0, ADD, MAX)
                ve.wait_ge(swr, 32)
                for kind, t in DVE_ORDER:
                    st = t % 2
                    if kind == "h2":
                        ve.wait_ge(s2, POS_PE[("B", t)])
                        ve.tensor_scalar(
                            h2sb[:, st, :], ps2[:], b2v[:], 0.0, ADD, MAX
                        ).then_inc(sv, 1)
                    else:
                        ve.wait_ge(s2, POS_PE[("D", t)])
                        ve.tensor_scalar(
                            osb[:, t, :], ps4[:], b4v[:], None, ADD
                        ).then_inc(sv, 1)

            @block.tensor
            def _(te):
                te.wait_ge(sg, 1)
                for _i in range(N_WARM_MM):
                    te.matmul(ps2[0:1, :], warm[:, 0:1], warm[:, 1:513],
                              start=True, stop=True)

                def emit_A(t, m):
                    st = t % 2
                    if t >= 2:
                        # ps1[st, m] freed once r(t-2, m) consumed it
                        te.wait_ge(sa, POS_A[("r", t - 2, m)])
                    for c in range(NKC):
                        if m == 0:
                            if t == 0:
                                for g, (a, _b) in enumerate(X0_SPLITS):
                                    if a == c:
                                        te.wait_ge(sx0[g], 16)
                                if c == W1A[0]:
                                    te.wait_ge(sw1a, 16)
                                if c == W1B[0]:
                                    te.wait_ge(sw1b, 16)
                            else:
                                for g, (cg0, _cg1) in enumerate(XT_SPLITS):
                                    if cg0 == c:
                                        te.wait_ge(sxt[t][g], 16)
                        mm = te.matmul(
                            ps1[:, st, m, :],
                            w1sb[:, c, m * 128 : (m + 1) * 128],
                            xsb[:, t, c, :],
                            start=(c == 0),
                            stop=(c == NKC - 1),
                        )
                        if c == NKC - 1:
                            mm.then_inc(sm, 1)

                def emit_B(t):
                    st = t % 2
                    if t == 0:
                        te.wait_ge(swr, 32)
                    if t >= 1:
                        te.wait_ge(sv, POS_V[("h2", t - 1)])  # ps2 free
                    te.wait_ge(sa, POS_A[("r", t, 0)])
                    te.matmul(
                        ps2[:], w2v[:, 0, :], h1sb[:, st, 0, :],
                        start=True, stop=False,
                    )
                    te.wait_ge(sa, POS_A[("r", t, 1)])
                    te.matmul(
                        ps2[:], w2v[:, 1, :], h1sb[:, st, 1, :],
                        start=False, stop=True,
                    ).then_inc(s2, 1)

                def emit_C(t):
                    st = t % 2
                    if t >= 1:
                        te.wait_ge(sa, POS_A[("h3", t - 1)])  # ps3 free
                    te.wait_ge(sv, POS_V[("h2", t)])
                    te.matmul(
                        ps3[:], w3v[:], h2sb[:, st, :], start=True, stop=True
                    ).then_inc(s2, 1)

                def emit_D(t):
                    st = t % 2
                    if t >= 1:
                        te.wait_ge(sv, POS_V[("out", t - 1)])  # ps4 free
                    te.wait_ge(sa, POS_A[("h3", t)])
                    te.matmul(
                        ps4[:], w4v[:], h3sb[:, st, :], start=True, stop=True
                    ).then_inc(s2, 1)

                emit_A(0, 0)
                emit_A(0, 1)
                emit_A(1, 0)
                emit_A(1, 1)
                emit_B(0)
                emit_A(2, 0)
                emit_C(0)
                emit_A(2, 1)
                emit_B(1)
                emit_D(0)
                emit_A(3, 0)
                emit_C(1)
                emit_B(2)
                emit_A(3, 1)
                emit_D(1)
                emit_C(2)
                emit_B(3)
                emit_D(2)
                emit_C(3)
                emit_D(3)

        # Post-block: only Sync still has work (awaiting out-DMA landing in
        # HBM).  Walrus's per-engine semaphore-zeroing epilogue chains start
        # right after each engine's barrier release, overlapping this wait.
        nc.sync.wait_ge(sof, 16 * NT)

    return nc


def _np_dt(dt):
    if dt == BF16:
        return ml_dtypes.bfloat16
    if dt == FP16:
        return np.float16
    return np.float32


def prepare_inputs(x, conv_w, w1, b1, w2, b2, w3, b3, w4, b4,
                   l1_dt=FP16, l234_dt=FP16):
    w1v = np.ascontiguousarray(w1.T).reshape(26, 26, 256)
    w1e = np.zeros((28, 28, 256), dtype=np.float32)
    for di in range(3):
        for dj in range(3):
            w1e[di : di + 26, dj : dj + 26, :] += conv_w[di, dj] * w1v
    w1e = w1e.reshape(784, 256)
    w1t = np.ascontiguousarray(
        w1e.reshape(NKC, KC, 256).transpose(1, 0, 2)
    ).reshape(KC, NKC * 256).astype(_np_dt(l1_dt))

    w2t = np.ascontiguousarray(w2.T).reshape(2, 128, 128).transpose(1, 0, 2)
    wpack = np.zeros((128, 256 + 64 + 10), dtype=np.float32)
    wpack[:, 0:256] = w2t.reshape(128, 256)
    wpack[:, 256:320] = w3.T
    wpack[0:64, 320:330] = w4.T
    wpack = wpack.astype(_np_dt(l234_dt))

    bpack = np.zeros((128, 5), dtype=np.float32)
    bpack[:, 0:2] = b1.reshape(2, 128).T
    bpack[:, 2] = b2
    bpack[0:64, 3] = b3
    bpack[0:10, 4] = b4

    shared = {"w1e": w1t, "wpack": wpack, "bpack": bpack}
    in_maps = []
    for m in range(N_CORES):
        xc = x[m * BC : (m + 1) * BC]
        xt = np.ascontiguousarray(
            xc.reshape(NT, NB, NKC, KC).transpose(0, 3, 2, 1)
        ).reshape(NT, KC, NKC * NB).astype(_np_dt(l1_dt))
        in_maps.append({"xt": xt, **shared})
    return in_maps



_PROGRAM = None


def _get_program():
    global _PROGRAM
    if _PROGRAM is None:
        _PROGRAM = build_program()
    return _PROGRAM


def kernel(x, conv_w, w1, b1, w2, b2, w3, b3, w4, b4):
    from concourse import bass_utils

    args = [x, conv_w, w1, b1, w2, b2, w3, b3, w4, b4]
    x, conv_w, w1, b1, w2, b2, w3, b3, w4, b4 = [
        np.asarray(a, dtype=np.float32) for a in args
    ]
    nc = _get_program()
    in_maps = prepare_inputs(x, conv_w, w1, b1, w2, b2, w3, b3, w4, b4)
    res = bass_utils.run_bass_kernel_spmd(nc, in_maps, list(range(N_CORES)))
    out = np.concatenate(
        [np.ascontiguousarray(res.results[m]["outT"].T) for m in range(N_CORES)],
        axis=0,
    )
    return out.astype(np.float32)


# revision 13
# speedup vs baseline: 1.0602x; 1.0034x over previous
"""Trainium2 Bass kernel for nn_DigitConvolutionalModel (dense CNN -> MLP).

Pure data parallel over 8 NeuronCores (2048 samples each). The 3x3 conv is
linear, so the host folds it into the first FC layer (W1e = C @ w1.T), making
the whole network a 4-layer MLP computed in transposed orientation (features
on partitions, batch on the free dim) in fp16 (psum fp32, ~5e-4 rel err):

    outT = w4t.T @ relu(w3t.T @ relu(w2t.T @ relu(W1e.T @ xT + b1) + b2) + b3) + b4

v2 structure (vs the single-queue baseline):
  - Input DMA split over THREE parallel queues: SP (sync, HWDGE),
    ACT (scalar, HWDGE) and Pool (gpsimd, SWDGE); each x tile split into
    chunk groups c0-2 / c3-4 / c5-6, one group per queue, tile-ordered.
  - PE warmup matmuls sized to keep PE busy until the first x chunks land
    (HAM flips to 2.4 GHz during A0 instead of during A2).
  - PE tail ops pulled forward: C1 B2 between A3's m-groups, only the
    B3->C3->D3 chain of the last tile is exposed after A3.
  - All kernel semaphores pinned to nums 208+ (the Sync engine's chunk of
    walrus's end-of-NEFF semaphore-zeroing epilogue) and the final
    out-DMA wait moved AFTER the block barrier, so the ~6us per-engine
    zeroing chains overlap the out-DMA completion instead of following it.

PE op order (A=L1 m-group, B=L2, C=L3, D=L4):
  A0m0 A0m1 A1m0 A1m1 B0 A2m0 C0 A2m1 B1 D0 A3m0 C1 B2 A3m1 D1 C2 B3 D2 C3 D3
ACT: r00 r01 r10 r11 r20 r21 h3(0) r30 r31 h3(1) h3(2) h3(3)   (sa +1 each)
DVE: h2(0) h2(1) out(0) h2(2) out(1) h2(3) out(2) out(3)       (sv +1 each)
s2 counts PE tail ops (B/C/D) in PE order.
"""

from contextlib import ExitStack

import ml_dtypes
import numpy as np

import concourse.bass as bass
import concourse.mybir as mybir

N_CORES = 8
B = 16384
BC = B // N_CORES
NB = 512
NT = BC // NB
KC = 112
NKC = 7

F32 = mybir.dt.float32
BF16 = mybir.dt.bfloat16
FP16 = mybir.dt.float16
RELU = mybir.ActivationFunctionType.Relu
ADD = mybir.AluOpType.add
MAX = mybir.AluOpType.max

N_WARM_MM = 9

# x chunk splits per tile on the single sync DMA queue in need order:
# tile 0 fine-grained so A0 starts as chunks land, tiles 1-3 in two chunks.
X0_SPLITS = [(0, 1), (1, 2), (2, 4), (4, 7)]
XT_SPLITS = [(0, 4), (4, 7)]
W1A = (0, 3)
W1B = (3, 7)

# PE tail-op order; s2 threshold = 1-based position.
TAIL_ORDER = [
    ("B", 0), ("C", 0), ("B", 1), ("D", 0), ("C", 1), ("B", 2),
    ("D", 1), ("C", 2), ("B", 3), ("D", 2), ("C", 3), ("D", 3),
]
POS_PE = {op: i + 1 for i, op in enumerate(TAIL_ORDER)}

ACT_ORDER = [
    ("r", 0, 0), ("r", 0, 1), ("r", 1, 0), ("r", 1, 1), ("r", 2, 0),
    ("r", 2, 1), ("h3", 0), ("r", 3, 0), ("h3", 1), ("r", 3, 1),
    ("h3", 2), ("h3", 3),
]
POS_A = {op: i + 1 for i, op in enumerate(ACT_ORDER)}

DVE_ORDER = [
    ("h2", 0), ("h2", 1), ("out", 0), ("h2", 2), ("out", 1), ("h2", 3),
    ("out", 2), ("out", 3),
]
POS_V = {op: i + 1 for i, op in enumerate(DVE_ORDER)}

# Explicit semaphore numbers inside [208, 255]: the chunk of walrus's
# end-of-NEFF zeroing epilogue that the Sync engine clears LAST (after its
# own final instruction, i.e. after the post-block sof wait).  Any sem a
# post-barrier wait depends on must live here.
SEM_BASE = 208


def build_program(l1_dt=FP16, l234_dt=FP16):
    nc = bass.Bass()

    n_wp = 256 + 64 + 10

    xt_d = nc.declare_dram_parameter("xt", [NT, KC, NKC * NB], l1_dt, isOutput=False)
    w1_d = nc.declare_dram_parameter("w1e", [KC, NKC * 256], l1_dt, isOutput=False)
    wp_d = nc.declare_dram_parameter("wpack", [128, n_wp], l234_dt, isOutput=False)
    bp_d = nc.declare_dram_parameter("bpack", [128, 5], F32, isOutput=False)
    out_d = nc.declare_dram_parameter("outT", [10, BC], F32, isOutput=True)

    ctx = ExitStack()
    with ctx:
        xsb = ctx.enter_context(nc.sbuf_tensor([KC, NT, NKC, NB], l1_dt))
        w1sb = ctx.enter_context(nc.sbuf_tensor([KC, NKC, 256], l1_dt))
        wpsb = ctx.enter_context(nc.sbuf_tensor([128, n_wp], l234_dt))
        bpsb = ctx.enter_context(nc.sbuf_tensor([128, 5], F32))
        h1sb = ctx.enter_context(nc.sbuf_tensor([128, 2, 2, NB], l234_dt))
        h2sb = ctx.enter_context(nc.sbuf_tensor([128, 2, NB], l234_dt))
        h3sb = ctx.enter_context(nc.sbuf_tensor([64, 2, NB], l234_dt))
        osb = ctx.enter_context(nc.sbuf_tensor([10, NT, NB], F32))
        warm = ctx.enter_context(nc.sbuf_tensor([1, 513], BF16))
        dump_a = ctx.enter_context(nc.sbuf_tensor([1, 16], BF16))
        dump_v = ctx.enter_context(nc.sbuf_tensor([1, 16], BF16))

        w2v = wpsb[:, 0:256].rearrange("p (c o) -> p c o", c=2)
        w3v = wpsb[:, 256:320]
        w4v = wpsb[0:64, 320:330]
        b1v = bpsb[:, 0:2]
        b2v = bpsb[:, 2:3]
        b3v = bpsb[0:64, 3:4]
        b4v = bpsb[0:10, 4:5]

        ps1 = ctx.enter_context(nc.psum_tensor([128, 2, 2, NB], F32))
        ps2 = ctx.enter_context(nc.psum_tensor([128, NB], F32))
        ps3 = ctx.enter_context(nc.psum_tensor([64, NB], F32))
        ps4 = ctx.enter_context(nc.psum_tensor([10, NB], F32))

        nsem = iter(range(SEM_BASE, 256))

        def sem(name):
            return ctx.enter_context(nc.semaphore(name, num=next(nsem)))

        sg = sem("sg")
        swr = sem("swr")
        sw1a = sem("sw1a")
        sw1b = sem("sw1b")
        sx0 = [sem(f"sx0_{g}") for g in range(len(X0_SPLITS))]
        sxt = [None] + [
            [sem(f"sx{t}_{g}") for g in range(len(XT_SPLITS))]
            for t in range(1, NT)
        ]
        sm = sem("sm")
        s2 = sem("s2")
        sa = sem("sa")
        sv = sem("sv")
        sof = sem("sof")

        # Pre-block emission: input DMA issues and the PE warmup run right
        # after the framework preamble, ~1us before block-entry branches.
        a0, a1 = W1A
        nc.sync.dma_start(
            out=w1sb[:, a0:a1, :], in_=w1_d[:, a0 * 256 : a1 * 256]
        ).then_inc(sw1a, 16)
        for g in (0, 1):
            c0, c1 = X0_SPLITS[g]
            nc.sync.dma_start(
                out=xsb[:, 0, c0:c1, :], in_=xt_d[0, :, c0 * NB : c1 * NB]
            ).then_inc(sx0[g], 16)
        b0, b1 = W1B
        nc.sync.dma_start(
            out=w1sb[:, b0:b1, :], in_=w1_d[:, b0 * 256 : b1 * 256]
        ).then_inc(sw1b, 16)
        for g in (2, 3):
            c0, c1 = X0_SPLITS[g]
            nc.sync.dma_start(
                out=xsb[:, 0, c0:c1, :], in_=xt_d[0, :, c0 * NB : c1 * NB]
            ).then_inc(sx0[g], 16)
        for t in range(1, NT):
            for g, (c0, c1) in enumerate(XT_SPLITS):
                nc.sync.dma_start(
                    out=xsb[:, t, c0:c1, :], in_=xt_d[t, :, c0 * NB : c1 * NB]
                ).then_inc(sxt[t][g], 16)

        # warm tensor on gpsimd (fast memset) so PE needn't wait for DVE
        nc.gpsimd.memset(warm[:], 0.125).then_inc(sg, 1)
        nc.gpsimd.dma_start(out=wpsb[:], in_=wp_d[:]).then_inc(swr, 16)
        nc.gpsimd.dma_start(out=bpsb[:], in_=bp_d[:]).then_inc(swr, 16)

        with nc.Block() as block:

            @block.sync
            def _(sy):
                for t in range(NT):
                    sy.wait_ge(sv, POS_V[("out", t)])
                    sy.dma_start(
                        out=out_d[:, t * NB : (t + 1) * NB], in_=osb[:, t, :]
                    ).then_inc(sof, 16)

            @block.scalar
            def _(se):
                se.wait_ge(sg, 1)
                se.activation(dump_a[:], warm[:, 0:16], RELU)  # preload relu table
                se.wait_ge(swr, 32)
                for op in ACT_ORDER:
                    if op[0] == "r":
                        _, t, m = op
                        st = t % 2
                        if t >= 2:
                            # h1sb[st] freed once B(t-2) consumed it
                            se.wait_ge(s2, POS_PE[("B", t - 2)])
                        se.wait_ge(sm, 2 * t + m + 1)
                        se.activation(
                            h1sb[:, st, m, :], ps1[:, st, m, :], RELU,
                            bias=b1v[:, m : m + 1],
                        ).then_inc(sa, 1)
                    else:
                        _, t = op
                        st = t % 2
                        se.wait_ge(s2, POS_PE[("C", t)])
                        se.activation(
                            h3sb[:, st, :], ps3[:], RELU, bias=b3v[:]
                        ).then_inc(sa, 1)

            @block.vector
            def _(ve):
                ve.wait_ge(sg, 1)
                ve.tensor_scalar(dump_v[:], warm[:, 0:16], 0.0, 0.0, ADD, MAX)
                ve.wait_ge(swr, 32)
                for kind, t in DVE_ORDER:
                    st = t % 2
                    if kind == "h2":
                        ve.wait_ge(s2, POS_PE[("B", t)])
                        ve.tensor_scalar(
                            h2sb[:, st, :], ps2[:], b2v[:], 0.0, ADD, MAX
                        ).then_inc(sv, 1)
                    else:
                        ve.wait_ge(s2, POS_PE[("D", t)])
                        ve.tensor_scalar(
                            osb[:, t, :], ps4[:], b4v[:], None, ADD
                        ).then_inc(sv, 1)

            @block.tensor
            def _(te):
                te.wait_ge(sg, 1)

                def warm_mm(n):
                    # HAM keep-busy filler: self-contained scratch matmul
                    # into ps4[0:1] (D0's start=True overwrite comes later
                    # in PE program order, so this never races a reader).
                    for _i in range(n):
                        te.matmul(ps4[0:1, :], warm[:, 0:1], warm[:, 1:513],
                                  start=True, stop=True)

                warm_mm(N_WARM_MM)

                def emit_A(t, m, fillers=()):
                    st = t % 2
                    if t >= 2:
                        # ps1[st, m] freed once r(t-2, m) consumed it
                        te.wait_ge(sa, POS_A[("r", t - 2, m)])
                    for c in range(NKC):
                        if m == 0:
                            if t == 0:
                                for g, (a, _b) in enumerate(X0_SPLITS):
                                    if a == c:
                                        te.wait_ge(sx0[g], 16)
                                if c == W1A[0]:
                                    te.wait_ge(sw1a, 16)
                                if c == W1B[0]:
                                    te.wait_ge(sw1b, 16)
                            else:
                                for g, (cg0, _cg1) in enumerate(XT_SPLITS):
                                    if cg0 == c:
                                        te.wait_ge(sxt[t][g], 16)
                        mm = te.matmul(
                            ps1[:, st, m, :],
                            w1sb[:, c, m * 128 : (m + 1) * 128],
                            xsb[:, t, c, :],
                            start=(c == 0),
                            stop=(c == NKC - 1),
                        )
                        if c == NKC - 1:
                            mm.then_inc(sm, 1)
                        for fc, fn in fillers:
                            if fc == c:
                                warm_mm(fn)

                def emit_A_interleaved(t):
                    # both m-groups advance chunk-by-chunk so the tile
                    # finishes ~2 matmuls after its last chunk lands
                    st = t % 2
                    te.wait_ge(sa, POS_A[("r", t - 2, 1)])
                    for c in range(NKC):
                        for g, (cg0, _cg1) in enumerate(XT_SPLITS):
                            if cg0 == c:
                                te.wait_ge(sxt[t][g], 16)
                        for m in range(2):
                            mm = te.matmul(
                                ps1[:, st, m, :],
                                w1sb[:, c, m * 128 : (m + 1) * 128],
                                xsb[:, t, c, :],
                                start=(c == 0),
                                stop=(c == NKC - 1),
                            )
                            if c == NKC - 1:
                                mm.then_inc(sm, 1)

                def emit_B(t):
                    st = t % 2
                    if t == 0:
                        te.wait_ge(swr, 32)
                    if t >= 1:
                        te.wait_ge(sv, POS_V[("h2", t - 1)])  # ps2 free
                    te.wait_ge(sa, POS_A[("r", t, 0)])
                    te.matmul(
                        ps2[:], w2v[:, 0, :], h1sb[:, st, 0, :],
                        start=True, stop=False,
                    )
                    te.wait_ge(sa, POS_A[("r", t, 1)])
                    te.matmul(
                        ps2[:], w2v[:, 1, :], h1sb[:, st, 1, :],
                        start=False, stop=True,
                    ).then_inc(s2, 1)

                def emit_C(t):
                    st = t % 2
                    if t >= 1:
                        te.wait_ge(sa, POS_A[("h3", t - 1)])  # ps3 free
                    te.wait_ge(sv, POS_V[("h2", t)])
                    te.matmul(
                        ps3[:], w3v[:], h2sb[:, st, :], start=True, stop=True
                    ).then_inc(s2, 1)

                def emit_D(t):
                    st = t % 2
                    if t >= 1:
                        te.wait_ge(sv, POS_V[("out", t - 1)])  # ps4 free
                    te.wait_ge(sa, POS_A[("h3", t)])
                    te.matmul(
                        ps4[:], w4v[:], h3sb[:, st, :], start=True, stop=True
                    ).then_inc(s2, 1)

                emit_A(0, 0, fillers=[(1, 2), (3, 3)])
                emit_A(0, 1)
                emit_A(1, 0, fillers=[(3, 1)])
                emit_A(1, 1)
                emit_B(0)
                emit_A(2, 0, fillers=[(3, 1)])
                emit_C(0)
                emit_A(2, 1)
                emit_B(1)
                emit_A_interleaved(3)
                emit_D(0)
                emit_C(1)
                emit_B(2)
                emit_D(1)
                emit_C(2)
                emit_B(3)
                emit_D(2)
                emit_C(3)
                emit_D(3)

        # Post-block: only Sync still has work (awaiting out-DMA landing in
        # HBM).  Walrus's per-engine semaphore-zeroing epilogue chains start
        # right after each engine's barrier release, overlapping this wait.
        nc.sync.wait_ge(sof, 16 * NT)

    return nc


def _np_dt(dt):
    if dt == BF16:
        return ml_dtypes.bfloat16
    if dt == FP16:
        return np.float16
    return np.float32


def prepare_inputs(x, conv_w, w1, b1, w2, b2, w3, b3, w4, b4,
                   l1_dt=FP16, l234_dt=FP16):
    w1v = np.ascontiguousarray(w1.T).reshape(26, 26, 256)
    w1e = np.zeros((28, 28, 256), dtype=np.float32)
    for di in range(3):
        for dj in range(3):
            w1e[di : di + 26, dj : dj + 26, :] += conv_w[di, dj] * w1v
    w1e = w1e.reshape(784, 256)
    w1t = np.ascontiguousarray(
        w1e.reshape(NKC, KC, 256).transpose(1, 0, 2)
    ).reshape(KC, NKC * 256).astype(_np_dt(l1_dt))

    w2t = np.ascontiguousarray(w2.T).reshape(2, 128, 128).transpose(1, 0, 2)
    wpack = np.zeros((128, 256 + 64 + 10), dtype=np.float32)
    wpack[:, 0:256] = w2t.reshape(128, 256)
    wpack[:, 256:320] = w3.T
    wpack[0:64, 320:330] = w4.T
    wpack = wpack.astype(_np_dt(l234_dt))

    bpack = np.zeros((128, 5), dtype=np.float32)
    bpack[:, 0:2] = b1.reshape(2, 128).T
    bpack[:, 2] = b2
    bpack[0:64, 3] = b3
    bpack[0:10, 4] = b4

    shared = {"w1e": w1t, "wpack": wpack, "bpack": bpack}
    in_maps = []
    for m in range(N_CORES):
        xc = x[m * BC : (m + 1) * BC]
        xt = np.ascontiguousarray(
            xc.reshape(NT, NB, NKC, KC).transpose(0, 3, 2, 1)
        ).reshape(NT, KC, NKC * NB).astype(_np_dt(l1_dt))
        in_maps.append({"xt": xt, **shared})
    return in_maps



_PROGRAM = None


def _get_program():
    global _PROGRAM
    if _PROGRAM is None:
        _PROGRAM = build_program()
    return _PROGRAM


def kernel(x, conv_w, w1, b1, w2, b2, w3, b3, w4, b4):
    from concourse import bass_utils

    args = [x, conv_w, w1, b1, w2, b2, w3, b3, w4, b4]
    x, conv_w, w1, b1, w2, b2, w3, b3, w4, b4 = [
        np.asarray(a, dtype=np.float32) for a in args
    ]
    nc = _get_program()
    in_maps = prepare_inputs(x, conv_w, w1, b1, w2, b2, w3, b3, w4, b4)
    res = bass_utils.run_bass_kernel_spmd(nc, in_maps, list(range(N_CORES)))
    out = np.concatenate(
        [np.ascontiguousarray(res.results[m]["outT"].T) for m in range(N_CORES)],
        axis=0,
    )
    return out.astype(np.float32)
